# revision 1
# baseline (speedup 1.0000x reference)
import sys

sys.path.insert(0, "/opt/trn_rl_repo")
from contextlib import ExitStack

import numpy as np

import concourse.bass as bass
import concourse.mybir as mybir
import concourse.tile as tile
from concourse import bacc

# ---- problem constants (hardcoded; must match reference.py) ----
B, C, IMG = 2, 96, 256
WS = 2
NS = IMG // WS          # 128 patches per side
N = NS * NS             # 16384 tokens
TD = C * WS * WS        # 384 token dim
H = 6                   # heads
D = TD // H             # 64 head dim
W1 = 128                # one-sided window
G = 50                  # global tokens
NCORES = 8
SPLITS = 4              # sequence splits per batch
QLEN = N // SPLITS      # 4096 queries per core
NCH = QLEN // W1        # 32 query chunks per core
HALO = W1
NTOK = QLEN + 2 * HALO  # 4352 tokens incl halo
KCH = NCH + 2           # 34 key chunks incl halo
GPOS = np.linspace(0, N - 1, G).astype(np.int32)

_cache = {}


ABLATE = set()  # {"gq", "band", "pv", "proj_qk"} for sim experiments


def _build_program(reps=1):
    f32 = mybir.dt.float32
    f16 = mybir.dt.float16
    AF = mybir.ActivationFunctionType
    nc = bacc.Bacc("TRN2", target_bir_lowering=False, debug=False,
                   num_devices=NCORES)

    # ---- DRAM I/O ----
    tokT_d = nc.dram_tensor("tokT", [TD, NTOK], f16, kind="ExternalInput")
    tokgT_d = nc.dram_tensor("tokgT", [TD, G], f16, kind="ExternalInput")
    wnames = ["wq", "wk", "wv", "wkg", "wvg", "wqg"]
    w_d = {nm: nc.dram_tensor(nm, [TD, TD], f16, kind="ExternalInput")
           for nm in wnames}
    # f32 per-partition biases (q/k/kga/qg); fp16 row biases (v, vga)
    bnames = ["bq", "bk", "bkg", "bqg"]
    b_d = {nm: nc.dram_tensor(nm, [TD], f32, kind="ExternalInput")
           for nm in bnames}
    bvh_d = nc.dram_tensor("bvh", [TD], f16, kind="ExternalInput")
    bvgh_d = nc.dram_tensor("bvgh", [TD], f16, kind="ExternalInput")
    # masks: quad [g0(triu), g2(tril)] x 2 heads = [128, 512] fp16
    m_d = {nm: nc.dram_tensor(nm, [W1, 4 * W1], f16, kind="ExternalInput")
           for nm in ["m_std", "m_first", "m_last"]}
    # outputs: transposed attention out (with denominator row 64), og partials
    out_d = nc.dram_tensor("out_t", [H, D + 1, QLEN], f32,
                           kind="ExternalOutput")
    og_d = nc.dram_tensor("og_part", [H, G, D + 1], f32, kind="ExternalOutput")

    with tile.TileContext(nc) as tc, ExitStack() as ctx:
        const = ctx.enter_context(tc.tile_pool(name="const", bufs=1))
        tokp = ctx.enter_context(tc.tile_pool(name="tokp", bufs=1))
        vp = ctx.enter_context(tc.tile_pool(name="vp", bufs=1))
        pairp = ctx.enter_context(tc.tile_pool(name="pairp", bufs=2))
        pp = ctx.enter_context(tc.tile_pool(name="pp", bufs=5))
        outp = ctx.enter_context(tc.tile_pool(name="outp", bufs=4))
        psA = ctx.enter_context(tc.tile_pool(name="psA", bufs=2, space="PSUM"))
        psS = ctx.enter_context(tc.tile_pool(name="psS", bufs=2, space="PSUM"))
        psO = ctx.enter_context(tc.tile_pool(name="psO", bufs=2, space="PSUM"))

        # ---- constants into SBUF ----
        w_sb = {}
        for nm in wnames:
            t = const.tile([128, 3, TD], f16, name=f"{nm}_sb")
            nc.sync.dma_start(
                out=t, in_=w_d[nm].ap().rearrange("(kj p) f -> p kj f", p=128))
            w_sb[nm] = t
        b_sb = {}
        for nm in bnames:
            t = const.tile([128, 3], f32, name=f"{nm}_sb")
            nc.sync.dma_start(
                out=t, in_=b_d[nm].ap().rearrange("(m p) -> p m", p=128))
            b_sb[nm] = t
        def asrow(ap):
            return bass.AP(tensor=ap.tensor, offset=ap.offset,
                           ap=[[0, 1]] + list(ap.ap))

        bvh_sb = const.tile([1, TD], f16, name="bvh_sb")
        nc.sync.dma_start(out=bvh_sb, in_=asrow(bvh_d.ap()))
        bvgh_sb = const.tile([1, TD], f16, name="bvgh_sb")
        nc.sync.dma_start(out=bvgh_sb, in_=asrow(bvgh_d.ap()))
        ones_sb = const.tile([1, 128], f16, name="ones_sb")
        nc.vector.memset(ones_sb, 1.0)
        m_sb = {}
        for nm in m_d:
            t = const.tile([W1, 4 * W1], f16, name=f"{nm}_sb")
            nc.sync.dma_start(out=t, in_=m_d[nm][:, :])
            m_sb[nm] = t
        tokgT_sb = const.tile([128, 3, G], f16, name="tokgT_sb")
        for mi in range(3):
            nc.sync.dma_start(out=tokgT_sb[:, mi, :],
                              in_=tokgT_d[mi * 128:(mi + 1) * 128, :])
        tokT_sb = tokp.tile([128, 3, NTOK], f16, name="tokT_sb")
        for mi in range(3):
            nc.sync.dma_start(out=tokT_sb[:, mi, :],
                              in_=tokT_d[mi * 128:(mi + 1) * 128, :])

        # ---- compute body (repeatable for benchmarking) ----
        if reps > 1:
            loop_ctx = tc.For_i(0, reps, 1)
            loop_ctx.__enter__()
        for _rep in range(1):
            # global-token projections: qgT (Wqg), kgT (Wk), vg_aug (Wv)
            qgT_sb = vp.tile([128, 3, G], f16, name="qgT_sb", tag="qgT")
            kgT_sb = vp.tile([128, 3, 128], f16, name="kgT_sb", tag="kgT")
            vg_aug = vp.tile([128, H, D + 1], f16, name="vg_aug", tag="vgaug")
            nc.vector.memset(kgT_sb, 0.0)
            nc.vector.memset(vg_aug, 0.0)
            for mi in range(3):
                ms = slice(mi * 128, (mi + 1) * 128)
                ps_q = psA.tile([128, 512], f32, name="ps_gq", tag="pj")
                for kj in range(3):
                    nc.tensor.matmul(ps_q[:, 0:G], lhsT=w_sb["wqg"][:, kj, ms],
                                     rhs=tokgT_sb[:, kj, :],
                                     start=kj == 0, stop=kj == 2)
                nc.vector.tensor_scalar_add(qgT_sb[:, mi, :], ps_q[:, 0:G],
                                            b_sb["bqg"][:, mi:mi + 1])
                ps_k = psA.tile([128, 512], f32, name="ps_gk", tag="pj")
                for kj in range(3):
                    nc.tensor.matmul(ps_k[:, 0:G], lhsT=w_sb["wk"][:, kj, ms],
                                     rhs=tokgT_sb[:, kj, :],
                                     start=kj == 0, stop=kj == 2)
                nc.vector.tensor_scalar_add(kgT_sb[:, mi, 0:G], ps_k[:, 0:G],
                                            b_sb["bk"][:, mi:mi + 1])
            ps_vg = psA.tile([128, 512], f32, name="ps_vg", tag="pj")
            for kj in range(3):
                nc.tensor.matmul(ps_vg[0:G, 0:TD], lhsT=tokgT_sb[:, kj, :],
                                 rhs=w_sb["wv"][:, kj, :],
                                 start=kj == 0, stop=False)
            nc.tensor.matmul(ps_vg[0:G, 0:TD], lhsT=ones_sb[:, 0:G],
                             rhs=bvh_sb, start=False, stop=True)
            nc.vector.tensor_copy(
                vg_aug[0:G, :, 0:D],
                ps_vg[0:G, 0:TD].rearrange("p (h d) -> p h d", h=H))
            nc.vector.memset(vg_aug[0:G, :, D:D + 1], 1.0)

            # v_all / vga_all: token-major, all heads, fp16, +ones column
            v_all = vp.tile([128, KCH, H, D + 1], f16, name="v_all",
                            tag="v_all")
            vga_all = vp.tile([128, NCH, H, D + 1], f16, name="vga_all",
                              tag="vga_all")
            for (dst, wname, brow, nch, toff) in (
                    (v_all, "wv", bvh_sb, KCH, 0),
                    (vga_all, "wvg", bvgh_sb, NCH, HALO)):
                for c in range(nch):
                    ps = psA.tile([128, 512], f32, name="ps_v", tag="pj")
                    for kj in range(3):
                        nc.tensor.matmul(
                            ps[:, 0:TD],
                            lhsT=tokT_sb[:, kj,
                                         toff + c * 128:toff + (c + 1) * 128],
                            rhs=w_sb[wname][:, kj, :],
                            start=kj == 0, stop=False)
                    nc.tensor.matmul(ps[:, 0:TD], lhsT=ones_sb, rhs=brow,
                                     start=False, stop=True)
                    nc.vector.tensor_copy(
                        dst[:, c, :, 0:D],
                        ps[:, 0:TD].rearrange("p (h d) -> p h d", h=H))
                nc.vector.memset(dst[:, :, :, D:D + 1], 1.0)

            # ---- per head-pair ----
            for j in range(3):
                js = slice(j * 128, (j + 1) * 128)
                qT = pairp.tile([128, QLEN], f16, name=f"qT{j}", tag="qT")
                kT = pairp.tile([128, NTOK], f16, name=f"kT{j}", tag="kT")
                kgaT = pairp.tile([128, QLEN], f16, name=f"kgaT{j}",
                                  tag="kgaT")

                for (dst, wname, bname, toff, ntk) in (
                        (qT, "wq", "bq", HALO, QLEN),
                        (kgaT, "wkg", "bkg", HALO, QLEN),
                        (kT, "wk", "bk", 0, NTOK)):
                    offs = [(ti * 512, min(512, ntk - ti * 512))
                            for ti in range((ntk + 511) // 512)]
                    for ti, (off, nn_) in enumerate(offs):
                        ps = psA.tile([128, 512], f32, name="ps_p", tag="pj")
                        for kj in range(3):
                            nc.tensor.matmul(
                                ps[:, 0:nn_], lhsT=w_sb[wname][:, kj, js],
                                rhs=tokT_sb[:, kj, toff + off:toff + off + nn_],
                                start=kj == 0, stop=kj == 2)
                        nc.vector.tensor_scalar_add(
                            dst[:, off:off + nn_], ps[:, 0:nn_],
                            b_sb[bname][:, j:j + 1])

                # ---- band + global scores by key-chunk; PV as ci completes --
                pT_live = {}

                def do_pv_pair(c0):
                    # outT[e, q] for query chunks (c0, c0+1), both heads.
                    # Adjacent chunks share v-chunks, so band PV merges into
                    # N=256 matmuls reading adjacent column groups of pT.
                    if "pv" in ABLATE:
                        return
                    ps_ot = psO.tile([D + 1, 512], f32, name="ps_ot",
                                     tag="ot")
                    for hh in range(2):
                        h = 2 * j + hh
                        ba = hh * 256
                        hf = hh * 512
                        nc.tensor.matmul(
                            ps_ot[:, ba:ba + 128],
                            lhsT=v_all[:, c0, h, :],
                            rhs=pT_live[c0][:, hf + 256:hf + 384],
                            start=True, stop=False)
                        nc.tensor.matmul(
                            ps_ot[:, ba:ba + 256],
                            lhsT=v_all[:, c0 + 1, h, :],
                            rhs=pT_live[c0 + 1][:, hf + 128:hf + 384],
                            start=False, stop=False)
                        nc.tensor.matmul(
                            ps_ot[:, ba:ba + 256],
                            lhsT=v_all[:, c0 + 2, h, :],
                            rhs=pT_live[c0 + 2][:, hf + 0:hf + 256],
                            start=False, stop=False)
                        nc.tensor.matmul(
                            ps_ot[:, ba + 128:ba + 256],
                            lhsT=v_all[:, c0 + 3, h, :],
                            rhs=pT_live[c0 + 3][:, hf + 0:hf + 128],
                            start=False, stop=False)
                        nc.tensor.matmul(
                            ps_ot[:, ba:ba + 128], lhsT=vg_aug[:, h, :],
                            rhs=pT_live[c0 + 2][:, hf + 384:hf + 512],
                            start=False, stop=False)
                        nc.tensor.matmul(
                            ps_ot[:, ba + 128:ba + 256],
                            lhsT=vg_aug[:, h, :],
                            rhs=pT_live[c0 + 3][:, hf + 384:hf + 512],
                            start=False, stop=True)
                    ot_sb = outp.tile([D + 1, 2, 256], f32, name="ot_sb",
                                      tag="ot_sb")
                    nc.vector.tensor_copy(
                        ot_sb, ps_ot.rearrange("e (h q) -> e h q", h=2))
                    nc.sync.dma_start(
                        out=out_d[2 * j:2 * j + 2, :,
                                  c0 * 128:(c0 + 2) * 128]
                        .rearrange("h e q -> e h q"),
                        in_=ot_sb)

                for kk in range(KCH) if "band" not in ABLATE else []:
                    qlo = max(kk - 2, 0)
                    qhi = min(kk, NCH - 1)
                    nq = qhi - qlo + 1
                    glo = 2 - (kk - qlo)  # first column group used
                    ps_s = psS.tile([128, 1024], f32, name="ps_s", tag="sT")
                    for hh in range(2):
                        hof = hh * 512
                        nc.tensor.matmul(
                            ps_s[:, hof + glo * 128:hof + (glo + nq) * 128],
                            lhsT=kT[hh * 64:hh * 64 + 64,
                                    kk * 128:(kk + 1) * 128],
                            rhs=qT[hh * 64:hh * 64 + 64,
                                   qlo * 128:(qhi + 1) * 128],
                            start=True, stop=True)
                        if kk >= 2:
                            ci = kk - 2
                            nc.tensor.matmul(
                                ps_s[:, hof + 384:hof + 512],
                                lhsT=kgT_sb[hh * 64:hh * 64 + 64, j, :],
                                rhs=qT[hh * 64:hh * 64 + 64,
                                       ci * 128:(ci + 1) * 128],
                                start=True, stop=True)
                    pt = pp.tile([128, 1024], f16, name="pT", tag="pT")
                    nc.scalar.activation(pt, ps_s, AF.Exp)
                    mt = m_sb["m_first"] if kk == 0 else (
                        m_sb["m_last"] if kk == KCH - 1 else m_sb["m_std"])
                    ptv = pt.rearrange("p (hh a b q) -> p hh a b q",
                                       hh=2, a=2, b=2)
                    mtv = mt.rearrange("p (hh a q) -> p hh a q", hh=2, a=2)
                    nc.vector.tensor_mul(ptv[:, :, :, 0, :],
                                         ptv[:, :, :, 0, :], mtv)
                    pT_live[kk] = pt
                    # chunk-pair PV fires once the 4 contributing pT exist
                    if kk >= 3 and kk % 2 == 1:
                        do_pv_pair(kk - 3)
                        del pT_live[kk - 3]
                        del pT_live[kk - 2]

                # ---- global-query attention (partial over own 4096 keys) ---
                if "gq" in ABLATE:
                    continue
                og_acc = outp.tile([G, 2, D + 1], f32, name="og_acc",
                                   tag="og_acc")
                # block-diagonal qg so one K=128 matmul scores both heads
                qg2 = outp.tile([128, 2 * G], f16, name="qg2", tag="qg2")
                nc.vector.memset(qg2, 0.0)
                nc.vector.tensor_copy(qg2[0:64, 0:G], qgT_sb[0:64, j, :])
                nc.vector.tensor_copy(qg2[64:128, G:2 * G],
                                      qgT_sb[64:128, j, :])
                groups = [list(range(gg * 10, min(gg * 10 + 10, NCH)))
                          for gg in range(4)]
                first = True
                for grp in groups:
                    ps_sg = psS.tile([128, 1024], f32, name="ps_sg", tag="sT")
                    for ii, ci in enumerate(grp):
                        nc.tensor.matmul(
                            ps_sg[:, ii * 100:ii * 100 + 100],
                            lhsT=kgaT[:, ci * 128:(ci + 1) * 128],
                            rhs=qg2, start=True, stop=True)
                    pg = pp.tile([128, 1024], f16, name="pg", tag="pT")
                    nc.scalar.activation(pg[:, 0:len(grp) * 100],
                                         ps_sg[:, 0:len(grp) * 100], AF.Exp)
                    for hh in range(2):
                        h = 2 * j + hh
                        ps_pv = psO.tile([G, 128], f32, name="ps_pv",
                                         tag="ot")
                        for ii, ci in enumerate(grp):
                            nc.tensor.matmul(
                                ps_pv[:, 0:D + 1],
                                lhsT=pg[:, ii * 100 + hh * G:
                                        ii * 100 + hh * G + G],
                                rhs=vga_all[:, ci, h, :],
                                start=ii == 0, stop=ii == len(grp) - 1)
                        if first:
                            nc.vector.tensor_copy(og_acc[:, hh, :],
                                                  ps_pv[:, 0:D + 1])
                        else:
                            nc.vector.tensor_add(og_acc[:, hh, :],
                                                 og_acc[:, hh, :],
                                                 ps_pv[:, 0:D + 1])
                    first = False
                for hh in range(2):
                    nc.sync.dma_start(out=og_d[2 * j + hh],
                                      in_=og_acc[:, hh, :])

        if reps > 1:
            loop_ctx.__exit__(None, None, None)

    nc.compile()
    return nc


def _get_exec(reps=1):
    """Build + jit the 8-core PJRT executable once per reps; cache it."""
    key = f"exec{reps}"
    if key in _cache:
        return _cache[key]
    import jax
    from jax.sharding import Mesh, PartitionSpec
    from jax.experimental.shard_map import shard_map
    from concourse import bass2jax
    import concourse.mybir as mybir_

    nc = _build_program(reps=reps)
    _cache[f"ncobj{reps}"] = nc
    bass2jax.install_neuronx_cc_hook()
    partition_name = (nc.partition_id_tensor.name
                      if nc.partition_id_tensor else None)
    in_names, out_names, out_avals, zero_shapes = [], [], [], []
    for alloc in nc.m.functions[0].allocations:
        if not isinstance(alloc, mybir_.MemoryLocationSet):
            continue
        name = alloc.memorylocations[0].name
        if alloc.kind == "ExternalInput":
            if name != partition_name:
                in_names.append(name)
        elif alloc.kind == "ExternalOutput":
            shape = tuple(alloc.tensor_shape)
            dtype = mybir_.dt.np(alloc.dtype)
            out_names.append(name)
            out_avals.append(jax.core.ShapedArray(shape, dtype))
            zero_shapes.append((shape, dtype))
    n_params = len(in_names)
    n_outs = len(out_avals)
    all_names = in_names + out_names
    if partition_name is not None:
        all_names = all_names + [partition_name]

    def _body(*args):
        operands = list(args)
        if partition_name is not None:
            operands.append(bass2jax.partition_id_tensor())
        outs = bass2jax._bass_exec_p.bind(
            *operands,
            out_avals=tuple(out_avals),
            in_names=tuple(all_names),
            out_names=tuple(out_names),
            lowering_input_output_aliases=(),
            sim_require_finite=True,
            sim_require_nnan=True,
            nc=nc,
        )
        return tuple(outs)

    donate = tuple(range(n_params, n_params + n_outs))
    devices = jax.devices()[:NCORES]
    mesh = Mesh(np.asarray(devices), ("core",))
    in_specs = (PartitionSpec("core"),) * (n_params + n_outs)
    out_specs = (PartitionSpec("core"),) * n_outs
    sharded = jax.jit(
        shard_map(_body, mesh=mesh, in_specs=in_specs, out_specs=out_specs,
                  check_rep=False),
        donate_argnums=donate, keep_unused=True)
    _cache[key] = (sharded, in_names, out_names, out_avals, zero_shapes)
    return _cache[key]


def _run(in_maps):
    sharded, in_names, out_names, out_avals, zero_shapes = _get_exec()
    concat_in = [
        np.concatenate([in_maps[c][nm] for c in range(NCORES)], axis=0)
        for nm in in_names]
    zeros = [np.zeros((NCORES * s[0], *s[1:]), dt) for s, dt in zero_shapes]
    out_arrs = sharded(*concat_in, *zeros)
    _cache["bench"] = (concat_in, zero_shapes)
    return [
        {nm: np.asarray(out_arrs[i]).reshape(NCORES, *out_avals[i].shape)[c]
         for i, nm in enumerate(out_names)}
        for c in range(NCORES)]


def bench_single(n=10, reps_list=(1, 3)):
    """Single-core timing: run the same SPMD body on device 0 only."""
    import time
    import jax
    from concourse import bass2jax

    concat_in, zero_shapes = _cache["bench"]
    out = {}
    for reps in reps_list:
        sharded, in_names, out_names, out_avals, zshapes = _get_exec(reps)
        # rebuild a single-device body using the same nc
        key = f"exec1core{reps}"
        if key not in _cache:
            nc = _cache[f"ncobj{reps}"]
            partition_name = (nc.partition_id_tensor.name
                              if nc.partition_id_tensor else None)
            all_names = list(in_names) + list(out_names)
            if partition_name is not None:
                all_names.append(partition_name)

            def _body(*args, _nc=nc, _all=tuple(all_names),
                      _outs=tuple(out_names), _avals=tuple(out_avals),
                      _pn=partition_name):
                operands = list(args)
                if _pn is not None:
                    operands.append(bass2jax.partition_id_tensor())
                return tuple(bass2jax._bass_exec_p.bind(
                    *operands, out_avals=_avals, in_names=_all,
                    out_names=_outs, lowering_input_output_aliases=(),
                    sim_require_finite=True, sim_require_nnan=True, nc=_nc))

            n_params = len(in_names)
            donate = tuple(range(n_params, n_params + len(out_names)))
            _cache[key] = jax.jit(_body, donate_argnums=donate,
                                  keep_unused=True)
        fn = _cache[key]
        dev0 = jax.devices()[0]
        per_core = [jax.device_put(a.reshape(NCORES, a.shape[0] // NCORES,
                                             *a.shape[1:])[0], dev0)
                    for a in concat_in]
        for a in per_core:
            a.block_until_ready()
        times = []
        for _ in range(n):
            zeros = [jax.device_put(np.zeros(s, dt), dev0)
                     for s, dt in zero_shapes]
            for z in zeros:
                z.block_until_ready()
            t0 = time.perf_counter()
            o = fn(*per_core, *zeros)
            for x in o:
                x.block_until_ready()
            times.append(time.perf_counter() - t0)
        out[reps] = times
    return out


def bench_calibrated(n=6, hi_reps=3):
    """Time reps=1 vs reps=hi_reps executables; slope = true per-body time.
    Requires kernel() to have been called first (for cached inputs)."""
    import time
    import jax

    concat_in, zero_shapes = _cache["bench"]
    dev_in = [jax.device_put(a) for a in concat_in]
    for a in dev_in:
        a.block_until_ready()

    def time_exec(reps):
        sharded = _get_exec(reps)[0]
        times = []
        for _ in range(n):
            zeros = [jax.device_put(np.zeros((NCORES * s[0], *s[1:]), dt))
                     for s, dt in zero_shapes]
            for z in zeros:
                z.block_until_ready()
            t0 = time.perf_counter()
            out = sharded(*dev_in, *zeros)
            for o in out:
                o.block_until_ready()
            times.append(time.perf_counter() - t0)
        return times

    t1 = time_exec(1)
    tR = time_exec(hi_reps)
    per = (min(tR) - min(t1)) / (hi_reps - 1)
    return t1, tR, per


def _tokens(x):
    b = x.shape[0]
    t = x.reshape(b, C, NS, WS, NS, WS).transpose(0, 1, 2, 4, 3, 5)
    t = t.reshape(b, C, N, WS * WS).transpose(0, 2, 1, 3)
    return np.ascontiguousarray(t.reshape(b, N, TD))


def _untokens(o):
    b = o.shape[0]
    o = o.reshape(b, NS, NS, C, WS, WS).transpose(0, 3, 1, 4, 2, 5)
    return np.ascontiguousarray(o.reshape(b, C, IMG, IMG))


def _make_masks(s):
    # quad mask [g0 | g2 | g0 | g2] as [128, 512]; g0=triu (q>=p), g2=tril
    triu = np.triu(np.ones((W1, W1), np.float16))
    tril = np.tril(np.ones((W1, W1), np.float16))
    zer = np.zeros((W1, W1), np.float16)
    std = np.concatenate([triu, tril, triu, tril], axis=1)
    first = std.copy()
    last = std.copy()
    if s == 0:  # global chunk 0: its block-0 (g2 slot of kk=0) is invalid
        first[:, 128:256] = zer
        first[:, 384:512] = zer
    if s == SPLITS - 1:  # global chunk 127: block-2 (g0 slot of kk=33) invalid
        last[:, 0:128] = zer
        last[:, 256:384] = zer
    return (np.ascontiguousarray(std), np.ascontiguousarray(first),
            np.ascontiguousarray(last))


def kernel(**inputs):
    x = np.asarray(inputs["x"], dtype=np.float32)
    tokens = _tokens(x)  # (B, N, TD)
    scale = np.float32(1.0 / np.sqrt(D))

    host_w = {
        "wq": np.asarray(inputs["Wq"], np.float32) * scale,
        "wk": np.asarray(inputs["Wk"], np.float32),
        "wv": np.asarray(inputs["Wv"], np.float32),
        "wkg": np.asarray(inputs["Wkg"], np.float32),
        "wvg": np.asarray(inputs["Wvg"], np.float32),
        "wqg": np.asarray(inputs["Wqg"], np.float32) * scale,
    }
    host_w = {k: np.ascontiguousarray(v.astype(np.float16))
              for k, v in host_w.items()}
    host_b = {
        "bq": np.asarray(inputs["bq"], np.float32) * scale,
        "bk": np.asarray(inputs["bk"], np.float32),
        "bkg": np.asarray(inputs["bkg"], np.float32),
        "bqg": np.asarray(inputs["bqg"], np.float32) * scale,
    }
    host_b = {k: np.ascontiguousarray(v) for k, v in host_b.items()}
    bvh = np.ascontiguousarray(
        np.asarray(inputs["bv"], np.float32).astype(np.float16))
    bvgh = np.ascontiguousarray(
        np.asarray(inputs["bvg"], np.float32).astype(np.float16))

    in_maps = []
    for core in range(NCORES):
        b, s = divmod(core, SPLITS)
        lo = s * QLEN - HALO
        hi = (s + 1) * QLEN + HALO
        shard = np.zeros((NTOK, TD), np.float32)
        s0, s1 = max(lo, 0), min(hi, N)
        shard[s0 - lo:s1 - lo] = tokens[b, s0:s1]
        tokT = np.ascontiguousarray(shard.T.astype(np.float16))
        tokgT = np.ascontiguousarray(
            tokens[b, GPOS].T.astype(np.float16))
        m_std, m_first, m_last = _make_masks(s)
        m = dict(host_w)
        m.update(host_b)
        m["bvh"] = bvh
        m["bvgh"] = bvgh
        m["tokT"] = tokT
        m["tokgT"] = tokgT
        m["m_std"] = m_std
        m["m_first"] = m_first
        m["m_last"] = m_last
        in_maps.append(m)

    results = _run(in_maps)

    out = np.empty((B, N, TD), np.float32)
    og_acc = np.zeros((B, H, G, D + 1), np.float64)
    for core in range(NCORES):
        b, s = divmod(core, SPLITS)
        arr = results[core]["out_t"]  # (H, D+1, QLEN)
        o = arr[:, :D, :] / arr[:, D:D + 1, :]  # (H, D, QLEN)
        out[b, s * QLEN:(s + 1) * QLEN] = (
            o.transpose(2, 0, 1).reshape(QLEN, TD))
        og_acc[b] += results[core]["og_part"]
    og = (og_acc[..., :D] / og_acc[..., D:D + 1]).astype(np.float32)
    og = og.transpose(0, 2, 1, 3).reshape(B, G, TD)  # (B, G, H*D)
    out[:, GPOS] = og
    return _untokens(out)



# revision 25
# speedup vs baseline: 2.7987x; 2.7987x over previous
import sys

sys.path.insert(0, "/opt/trn_rl_repo")
from contextlib import ExitStack

import numpy as np

import concourse.bass as bass
import concourse.mybir as mybir
import concourse.tile as tile
from concourse import bacc

# ---- problem constants (hardcoded; must match reference.py) ----
B, C, IMG = 2, 96, 256
WS = 2
NS = IMG // WS          # 128 patches per side
N = NS * NS             # 16384 tokens
TD = C * WS * WS        # 384 token dim
H = 6                   # heads
D = TD // H             # 64 head dim
W1 = 128                # one-sided window
G = 50                  # global tokens
NCORES = 8
SPLITS = 4              # sequence splits per batch
QLEN = N // SPLITS      # 4096 queries per core
NCH = QLEN // W1        # 32 query chunks per core
HALO = W1
NTOK = QLEN + 2 * HALO  # 4352 tokens incl halo
KCH = NCH + 2           # 34 key chunks incl halo
GPOS = np.linspace(0, N - 1, G).astype(np.int32)

_cache = {}


ABLATE = set()  # {"gq", "band", "pv", "proj_qk"} for sim experiments
MASK_POOL = False   # alternate interior masks DVE/GpSimd (else all DVE)
EXP_SHRINK = True   # shrink boundary-kk exp ops
REORDER_V = True    # emit j=0 projections before the V block
GQ_FIRST = False    # emit gq(j) before band(j) for j>=1
VEVAC_SPLIT = False  # alternate v-chunk evacuations DVE/ACT


def ecopy(nc, eng, out, in_):
    """PSUM->SBUF evacuation on either DVE (tensor_copy) or ACT
    (activation Copy); used to balance the two engines."""
    if eng is nc.vector:
        nc.vector.tensor_copy(out, in_)
    else:
        nc.scalar.activation(out, in_, mybir.ActivationFunctionType.Copy)


def _build_program(reps=1):
    f32 = mybir.dt.float32
    f16 = mybir.dt.float16
    AF = mybir.ActivationFunctionType
    nc = bacc.Bacc("TRN2", target_bir_lowering=False, debug=False,
                   num_devices=NCORES)

    # ---- DRAM I/O ----
    # Bias algebra: bk/bkg add a per-query constant to every score row
    # (q~.bk), which softmax cancels -- dropped entirely. bv/bvg shift the
    # attention output by a constant vector (since probs sum to 1) -- added
    # on the host after the denominator divide. Only bq/bqg remain on-device.
    tokT_d = nc.dram_tensor("tokT", [TD, NTOK], f16, kind="ExternalInput")
    tokgT_d = nc.dram_tensor("tokgT", [TD, G], f16, kind="ExternalInput")
    wnames = ["wq", "wk", "wv", "wkg", "wvg", "wqg"]
    w_d = {nm: nc.dram_tensor(nm, [TD, TD], f16, kind="ExternalInput")
           for nm in wnames}
    bnames = ["bq", "bqg"]
    b_d = {nm: nc.dram_tensor(nm, [TD], f32, kind="ExternalInput")
           for nm in bnames}
    # masks: quad [g0(triu), g2(tril)] x 2 heads = [128, 512] fp16
    m_d = {nm: nc.dram_tensor(nm, [W1, 4 * W1], f16, kind="ExternalInput")
           for nm in ["m_std", "m_first", "m_last"]}
    # outputs: transposed attention out (with denominator row 64), og partials
    out_d = nc.dram_tensor("out_t", [H, D + 1, QLEN], f32,
                           kind="ExternalOutput")
    og_d = nc.dram_tensor("og_part", [H, G, D + 1], f32, kind="ExternalOutput")

    with tile.TileContext(nc) as tc, ExitStack() as ctx:
        const = ctx.enter_context(tc.tile_pool(name="const", bufs=1))
        tokp = ctx.enter_context(tc.tile_pool(name="tokp", bufs=1))
        vp = ctx.enter_context(tc.tile_pool(name="vp", bufs=1))
        pairp = ctx.enter_context(tc.tile_pool(name="pairp", bufs=2))
        pp = ctx.enter_context(tc.tile_pool(name="pp", bufs=5))
        outp = ctx.enter_context(tc.tile_pool(name="outp", bufs=4))
        psA = ctx.enter_context(tc.tile_pool(name="psA", bufs=2, space="PSUM"))
        psS = ctx.enter_context(tc.tile_pool(name="psS", bufs=2, space="PSUM"))
        psO = ctx.enter_context(tc.tile_pool(name="psO", bufs=2, space="PSUM"))

        # ---- constants into SBUF ----
        w_sb = {}
        for nm in wnames:
            t = const.tile([128, 3, TD], f16, name=f"{nm}_sb")
            nc.sync.dma_start(
                out=t, in_=w_d[nm].ap().rearrange("(kj p) f -> p kj f", p=128))
            w_sb[nm] = t
        b_sb = {}
        for nm in bnames:
            t = const.tile([128, 3], f32, name=f"{nm}_sb")
            nc.sync.dma_start(
                out=t, in_=b_d[nm].ap().rearrange("(m p) -> p m", p=128))
            b_sb[nm] = t
        m_sb = {}
        for nm in m_d:
            t = const.tile([W1, 4 * W1], f16, name=f"{nm}_sb")
            nc.sync.dma_start(out=t, in_=m_d[nm][:, :])
            m_sb[nm] = t
        tokgT_sb = const.tile([128, 3, G], f16, name="tokgT_sb")
        for mi in range(3):
            nc.sync.dma_start(out=tokgT_sb[:, mi, :],
                              in_=tokgT_d[mi * 128:(mi + 1) * 128, :])
        tokT_sb = tokp.tile([128, 3, NTOK], f16, name="tokT_sb")
        for mi in range(3):
            nc.sync.dma_start(out=tokT_sb[:, mi, :],
                              in_=tokT_d[mi * 128:(mi + 1) * 128, :])

        # ---- compute body (repeatable for benchmarking) ----
        if reps > 1:
            loop_ctx = tc.For_i(0, reps, 1)
            loop_ctx.__enter__()
        for _rep in range(1):
            st = {}

            def emit_globals():
                # global-token projections: qgT (Wqg), kgT (Wk), vg_aug (Wv)
                qgT_sb = vp.tile([128, 3, G], f16, name="qgT_sb", tag="qgT")
                kgT_sb = vp.tile([128, 3, 128], f16, name="kgT_sb", tag="kgT")
                vg_aug = vp.tile([128, H, D + 1], f16, name="vg_aug",
                                 tag="vgaug")
                nc.vector.memset(kgT_sb, 0.0)
                nc.vector.memset(vg_aug, 0.0)
                for mi in range(3):
                    ms = slice(mi * 128, (mi + 1) * 128)
                    ps_q = psA.tile([128, 512], f32, name="ps_gq", tag="pj")
                    for kj in range(3):
                        nc.tensor.matmul(ps_q[:, 0:G],
                                         lhsT=w_sb["wqg"][:, kj, ms],
                                         rhs=tokgT_sb[:, kj, :],
                                         start=kj == 0, stop=kj == 2)
                    nc.vector.tensor_scalar_add(qgT_sb[:, mi, :],
                                                ps_q[:, 0:G],
                                                b_sb["bqg"][:, mi:mi + 1])
                    ps_k = psA.tile([128, 512], f32, name="ps_gk", tag="pj")
                    for kj in range(3):
                        nc.tensor.matmul(ps_k[:, 0:G],
                                         lhsT=w_sb["wk"][:, kj, ms],
                                         rhs=tokgT_sb[:, kj, :],
                                         start=kj == 0, stop=kj == 2)
                    nc.vector.tensor_copy(kgT_sb[:, mi, 0:G], ps_k[:, 0:G])
                ps_vg = psA.tile([128, 512], f32, name="ps_vg", tag="pj")
                for kj in range(3):
                    nc.tensor.matmul(ps_vg[0:G, 0:TD],
                                     lhsT=tokgT_sb[:, kj, :],
                                     rhs=w_sb["wv"][:, kj, :],
                                     start=kj == 0, stop=kj == 2)
                nc.vector.tensor_copy(
                    vg_aug[0:G, :, 0:D],
                    ps_vg[0:G, 0:TD].rearrange("p (h d) -> p h d", h=H))
                nc.vector.memset(vg_aug[0:G, :, D:D + 1], 1.0)
                st.update(qgT_sb=qgT_sb, kgT_sb=kgT_sb, vg_aug=vg_aug)

            def emit_vblock():
                # v_all / vga_all: token-major, all heads, fp16, +ones
                # column.  Returns one thunk per chunk so the caller can
                # interleave the projections into the band loop.
                v_all = vp.tile([128, KCH, H, D + 1], f16, name="v_all",
                                tag="v_all")
                vga_all = vp.tile([128, NCH, H, D + 1], f16, name="vga_all",
                                  tag="vga_all")
                thunks = []
                for (dst, wname, nch, toff) in (
                        (v_all, "wv", KCH, 0),
                        (vga_all, "wvg", NCH, HALO)):
                    nc.vector.memset(dst[:, :, :, D:D + 1], 1.0)

                    def th(dst=dst, wname=wname, toff=toff, c=0):
                        ps = psA.tile([128, 512], f32, name="ps_v", tag="pj")
                        for kj in range(3):
                            nc.tensor.matmul(
                                ps[:, 0:TD],
                                lhsT=tokT_sb[:, kj, toff + c * 128:
                                             toff + (c + 1) * 128],
                                rhs=w_sb[wname][:, kj, :],
                                start=kj == 0, stop=kj == 2)
                        eng = (nc.scalar if (VEVAC_SPLIT and c % 2 == 1)
                               else nc.vector)
                        ecopy(nc, eng,
                              dst[:, c, :, 0:D],
                              ps[:, 0:TD].rearrange("p (h d) -> p h d", h=H))
                    for c in range(nch):
                        thunks.append(lambda th=th, c=c: th(c=c))
                st.update(v_all=v_all, vga_all=vga_all)
                return thunks

            def emit_proj(j):
                js = slice(j * 128, (j + 1) * 128)
                qT = pairp.tile([128, QLEN], f16, name=f"qT{j}", tag="qT")
                kT = pairp.tile([128, NTOK], f16, name=f"kT{j}", tag="kT")
                kgaT = pairp.tile([128, QLEN], f16, name=f"kgaT{j}",
                                  tag="kgaT")
                thunks = []

                def tile_th(dst, wname, bname, toff, off, nn_):
                    ps = psA.tile([128, 512], f32, name="ps_p", tag="pj")
                    for kj in range(3):
                        nc.tensor.matmul(
                            ps[:, 0:nn_], lhsT=w_sb[wname][:, kj, js],
                            rhs=tokT_sb[:, kj,
                                        toff + off:toff + off + nn_],
                            start=kj == 0, stop=kj == 2)
                    if bname is not None:
                        nc.vector.tensor_scalar_add(
                            dst[:, off:off + nn_], ps[:, 0:nn_],
                            b_sb[bname][:, j:j + 1])
                    else:
                        ecopy(nc, nc.vector, dst[:, off:off + nn_],
                              ps[:, 0:nn_])

                for (dst, wname, bname, toff, ntk) in (
                        (qT, "wq", "bq", HALO, QLEN),
                        (kgaT, "wkg", None, HALO, QLEN),
                        (kT, "wk", None, 0, NTOK)):
                    for ti in range((ntk + 511) // 512):
                        off = ti * 512
                        nn_ = min(512, ntk - off)
                        thunks.append(
                            lambda a=dst, b=wname, c=bname, d=toff, e=off,
                            f=nn_: tile_th(a, b, c, d, e, f))
                return (qT, kT, kgaT), thunks

            def emit_band(j, qT, kT, kgaT, side=()):
                # `side`: deferred projection thunks drained into the kk
                # loop so PE/DVE stay fed while ACT works through the exps
                side = list(side)
                v_all, vg_aug = st["v_all"], st["vg_aug"]
                kgT_sb = st["kgT_sb"]
                nside = (len(side) + KCH - 1) // KCH if side else 0
                # band + global scores by key-chunk; PV as ci completes
                pT_live = {}

                def do_pv_pair(c0):
                    # outT[e, q] for query chunks (c0, c0+1), both heads.
                    # Adjacent chunks share v-chunks, so band PV merges into
                    # N=256 matmuls reading adjacent column groups of pT.
                    if "pv" in ABLATE:
                        return
                    ps_ot = psO.tile([D + 1, 512], f32, name="ps_ot",
                                     tag="ot")
                    for hh in range(2):
                        h = 2 * j + hh
                        ba = hh * 256
                        hf = hh * 512
                        nc.tensor.matmul(
                            ps_ot[:, ba:ba + 128],
                            lhsT=v_all[:, c0, h, :],
                            rhs=pT_live[c0][:, hf + 256:hf + 384],
                            start=True, stop=False)
                        nc.tensor.matmul(
                            ps_ot[:, ba:ba + 256],
                            lhsT=v_all[:, c0 + 1, h, :],
                            rhs=pT_live[c0 + 1][:, hf + 128:hf + 384],
                            start=False, stop=False)
                        nc.tensor.matmul(
                            ps_ot[:, ba:ba + 256],
                            lhsT=v_all[:, c0 + 2, h, :],
                            rhs=pT_live[c0 + 2][:, hf + 0:hf + 256],
                            start=False, stop=False)
                        nc.tensor.matmul(
                            ps_ot[:, ba + 128:ba + 256],
                            lhsT=v_all[:, c0 + 3, h, :],
                            rhs=pT_live[c0 + 3][:, hf + 0:hf + 128],
                            start=False, stop=False)
                        nc.tensor.matmul(
                            ps_ot[:, ba:ba + 128], lhsT=vg_aug[:, h, :],
                            rhs=pT_live[c0 + 2][:, hf + 384:hf + 512],
                            start=False, stop=False)
                        nc.tensor.matmul(
                            ps_ot[:, ba + 128:ba + 256],
                            lhsT=vg_aug[:, h, :],
                            rhs=pT_live[c0 + 3][:, hf + 384:hf + 512],
                            start=False, stop=True)
                    ot_sb = outp.tile([D + 1, 2, 256], f32, name="ot_sb",
                                      tag="ot_sb")
                    ecopy(nc, nc.vector, ot_sb,
                          ps_ot.rearrange("e (h q) -> e h q", h=2))
                    nc.sync.dma_start(
                        out=out_d[2 * j:2 * j + 2, :,
                                  c0 * 128:(c0 + 2) * 128]
                        .rearrange("h e q -> e h q"),
                        in_=ot_sb)

                for kk in range(KCH) if "band" not in ABLATE else []:
                    qlo = max(kk - 2, 0)
                    qhi = min(kk, NCH - 1)
                    nq = qhi - qlo + 1
                    glo = 2 - (kk - qlo)  # first column group used
                    ps_s = psS.tile([128, 1024], f32, name="ps_s", tag="sT")
                    for hh in range(2):
                        hof = hh * 512
                        nc.tensor.matmul(
                            ps_s[:, hof + glo * 128:hof + (glo + nq) * 128],
                            lhsT=kT[hh * 64:hh * 64 + 64,
                                    kk * 128:(kk + 1) * 128],
                            rhs=qT[hh * 64:hh * 64 + 64,
                                   qlo * 128:(qhi + 1) * 128],
                            start=True, stop=True)
                        if kk >= 2:
                            ci = kk - 2
                            nc.tensor.matmul(
                                ps_s[:, hof + 384:hof + 512],
                                lhsT=kgT_sb[hh * 64:hh * 64 + 64, j, :],
                                rhs=qT[hh * 64:hh * 64 + 64,
                                       ci * 128:(ci + 1) * 128],
                                start=True, stop=True)
                    pt = pp.tile([128, 1024], f16, name="pT", tag="pT")
                    # exp only the column ranges the PV stage actually reads;
                    # boundary kk touch a subset of the 4 col-groups per head
                    pse = ps_s.rearrange("p (hh c) -> p hh c", hh=2)
                    pte = pt.rearrange("p (hh c) -> p hh c", hh=2)
                    mq = lambda nm: m_sb[nm].rearrange(
                        "p (hh a q) -> p hh a q", hh=2, a=2)
                    if not EXP_SHRINK and 0 <= kk <= KCH - 1:
                        nc.scalar.activation(pt, ps_s, AF.Exp)
                        mt = m_sb["m_first"] if kk == 0 else (
                            m_sb["m_last"] if kk == KCH - 1 else m_sb["m_std"])
                        ptv = pt.rearrange("p (hh a b q) -> p hh a b q",
                                           hh=2, a=2, b=2)
                        mtv = mt.rearrange("p (hh a q) -> p hh a q",
                                           hh=2, a=2)
                        meng = (nc.vector if (kk % 2 == 0 or not MASK_POOL)
                                else nc.gpsimd)
                        meng.tensor_mul(ptv[:, :, :, 0, :],
                                        ptv[:, :, :, 0, :], mtv)
                    elif kk == 0:
                        nc.scalar.activation(pte[:, :, 256:384],
                                             pse[:, :, 256:384], AF.Exp)
                        nc.vector.tensor_mul(pte[:, :, 256:384],
                                             pte[:, :, 256:384],
                                             mq("m_first")[:, :, 1, :])
                    elif kk == 1:
                        nc.scalar.activation(pte[:, :, 128:384],
                                             pse[:, :, 128:384], AF.Exp)
                        nc.vector.tensor_mul(pte[:, :, 256:384],
                                             pte[:, :, 256:384],
                                             mq("m_std")[:, :, 1, :])
                    elif kk == KCH - 2:
                        nc.scalar.activation(pte[:, :, 0:256],
                                             pse[:, :, 0:256], AF.Exp)
                        nc.scalar.activation(pte[:, :, 384:512],
                                             pse[:, :, 384:512], AF.Exp)
                        nc.vector.tensor_mul(pte[:, :, 0:128],
                                             pte[:, :, 0:128],
                                             mq("m_std")[:, :, 0, :])
                    elif kk == KCH - 1:
                        nc.scalar.activation(pte[:, :, 0:128],
                                             pse[:, :, 0:128], AF.Exp)
                        nc.scalar.activation(pte[:, :, 384:512],
                                             pse[:, :, 384:512], AF.Exp)
                        nc.vector.tensor_mul(pte[:, :, 0:128],
                                             pte[:, :, 0:128],
                                             mq("m_last")[:, :, 0, :])
                    else:
                        nc.scalar.activation(pt, ps_s, AF.Exp)
                        ptv = pt.rearrange("p (hh a b q) -> p hh a b q",
                                           hh=2, a=2, b=2)
                        mtv = mq("m_std")
                        # alternate DVE / GpSimd to keep both off the
                        # critical path
                        meng = (nc.vector if (kk % 2 == 0 or not MASK_POOL)
                                else nc.gpsimd)
                        meng.tensor_mul(ptv[:, :, :, 0, :],
                                        ptv[:, :, :, 0, :], mtv)
                    pT_live[kk] = pt
                    for _ in range(nside):
                        if side:
                            side.pop(0)()
                    # chunk-pair PV fires once the 4 contributing pT exist
                    if kk >= 3 and kk % 2 == 1:
                        do_pv_pair(kk - 3)
                        del pT_live[kk - 3]
                        del pT_live[kk - 2]
                while side:
                    side.pop(0)()

            def emit_gq(j, kgaT):
                # global-query attention (partial over own 4096 keys)
                if "gq" in ABLATE:
                    return
                qgT_sb, vga_all = st["qgT_sb"], st["vga_all"]
                # block-diagonal qg so one K=128 matmul scores both heads
                qg2 = outp.tile([128, 2 * G], f16, name="qg2", tag="qg2")
                nc.vector.memset(qg2, 0.0)
                nc.vector.tensor_copy(qg2[0:64, 0:G], qgT_sb[0:64, j, :])
                nc.vector.tensor_copy(qg2[64:128, G:2 * G],
                                      qgT_sb[64:128, j, :])
                groups = [list(range(gg * 10, min(gg * 10 + 10, NCH)))
                          for gg in range(4)]
                # PV accumulates all 32 chunks into one PSUM tile per head
                ps_pv = {hh: psO.tile([G, 128], f32, name=f"ps_pv{hh}",
                                      tag="ot") for hh in range(2)}
                for grp in groups:
                    ps_sg = psS.tile([128, 1024], f32, name="ps_sg", tag="sT")
                    for ii, ci in enumerate(grp):
                        nc.tensor.matmul(
                            ps_sg[:, ii * 100:ii * 100 + 100],
                            lhsT=kgaT[:, ci * 128:(ci + 1) * 128],
                            rhs=qg2, start=True, stop=True)
                    pg = pp.tile([128, 1024], f16, name="pg", tag="pT")
                    nc.scalar.activation(pg[:, 0:len(grp) * 100],
                                         ps_sg[:, 0:len(grp) * 100], AF.Exp)
                    for hh in range(2):
                        h = 2 * j + hh
                        for ii, ci in enumerate(grp):
                            nc.tensor.matmul(
                                ps_pv[hh][:, 0:D + 1],
                                lhsT=pg[:, ii * 100 + hh * G:
                                        ii * 100 + hh * G + G],
                                rhs=vga_all[:, ci, h, :],
                                start=ci == 0, stop=ci == NCH - 1)
                og_sb = outp.tile([G, 2, D + 1], f32, name="og_sb",
                                  tag="og_acc")
                ecopy(nc, nc.vector, og_sb[:, 0, :], ps_pv[0][:, 0:D + 1])
                ecopy(nc, nc.vector, og_sb[:, 1, :], ps_pv[1][:, 0:D + 1])
                nc.sync.dma_start(
                    out=og_d[2 * j:2 * j + 2].rearrange("h g e -> g h e"),
                    in_=og_sb)

            # ---- emission order: j=0 Q/K projections emitted up front;
            # the V block and j+1 projections drain inside band(j)'s kk
            # loop so PE/DVE stay busy while ACT works the exps ----
            emit_globals()
            tiles0, th0 = emit_proj(0)
            for t in th0:
                t()
            vth = emit_vblock()
            tiles1, th1 = emit_proj(1)
            emit_band(0, *tiles0, side=vth + th1)
            emit_gq(0, tiles0[2])
            tiles2, th2 = emit_proj(2)
            if GQ_FIRST:
                emit_gq(1, tiles1[2])
                emit_band(1, *tiles1, side=th2)
                emit_gq(2, tiles2[2])
                emit_band(2, *tiles2)
            else:
                emit_band(1, *tiles1, side=th2)
                emit_gq(1, tiles1[2])
                emit_band(2, *tiles2)
                emit_gq(2, tiles2[2])

        if reps > 1:
            loop_ctx.__exit__(None, None, None)

    nc.compile()
    return nc


def _get_exec(reps=1):
    """Build + jit the 8-core PJRT executable once per reps; cache it."""
    key = f"exec{reps}"
    if key in _cache:
        return _cache[key]
    import jax
    from jax.sharding import Mesh, PartitionSpec
    from jax.experimental.shard_map import shard_map
    from concourse import bass2jax
    import concourse.mybir as mybir_

    nc = _build_program(reps=reps)
    _cache[f"ncobj{reps}"] = nc
    bass2jax.install_neuronx_cc_hook()
    partition_name = (nc.partition_id_tensor.name
                      if nc.partition_id_tensor else None)
    in_names, out_names, out_avals, zero_shapes = [], [], [], []
    for alloc in nc.m.functions[0].allocations:
        if not isinstance(alloc, mybir_.MemoryLocationSet):
            continue
        name = alloc.memorylocations[0].name
        if alloc.kind == "ExternalInput":
            if name != partition_name:
                in_names.append(name)
        elif alloc.kind == "ExternalOutput":
            shape = tuple(alloc.tensor_shape)
            dtype = mybir_.dt.np(alloc.dtype)
            out_names.append(name)
            out_avals.append(jax.core.ShapedArray(shape, dtype))
            zero_shapes.append((shape, dtype))
    n_params = len(in_names)
    n_outs = len(out_avals)
    all_names = in_names + out_names
    if partition_name is not None:
        all_names = all_names + [partition_name]

    def _body(*args):
        operands = list(args)
        if partition_name is not None:
            operands.append(bass2jax.partition_id_tensor())
        outs = bass2jax._bass_exec_p.bind(
            *operands,
            out_avals=tuple(out_avals),
            in_names=tuple(all_names),
            out_names=tuple(out_names),
            lowering_input_output_aliases=(),
            sim_require_finite=True,
            sim_require_nnan=True,
            nc=nc,
        )
        return tuple(outs)

    donate = tuple(range(n_params, n_params + n_outs))
    devices = jax.devices()[:NCORES]
    mesh = Mesh(np.asarray(devices), ("core",))
    in_specs = (PartitionSpec("core"),) * (n_params + n_outs)
    out_specs = (PartitionSpec("core"),) * n_outs
    sharded = jax.jit(
        shard_map(_body, mesh=mesh, in_specs=in_specs, out_specs=out_specs,
                  check_rep=False),
        donate_argnums=donate, keep_unused=True)
    _cache[key] = (sharded, in_names, out_names, out_avals, zero_shapes)
    return _cache[key]


def _run(in_maps):
    sharded, in_names, out_names, out_avals, zero_shapes = _get_exec()
    concat_in = [
        np.concatenate([in_maps[c][nm] for c in range(NCORES)], axis=0)
        for nm in in_names]
    zeros = [np.zeros((NCORES * s[0], *s[1:]), dt) for s, dt in zero_shapes]
    out_arrs = sharded(*concat_in, *zeros)
    _cache["bench"] = (concat_in, zero_shapes)
    return [
        {nm: np.asarray(out_arrs[i]).reshape(NCORES, *out_avals[i].shape)[c]
         for i, nm in enumerate(out_names)}
        for c in range(NCORES)]


def bench_single(n=10, reps_list=(1, 3)):
    """Single-core timing: run the same SPMD body on device 0 only."""
    import time
    import jax
    from concourse import bass2jax

    concat_in, zero_shapes = _cache["bench"]
    out = {}
    for reps in reps_list:
        sharded, in_names, out_names, out_avals, zshapes = _get_exec(reps)
        # rebuild a single-device body using the same nc
        key = f"exec1core{reps}"
        if key not in _cache:
            nc = _cache[f"ncobj{reps}"]
            partition_name = (nc.partition_id_tensor.name
                              if nc.partition_id_tensor else None)
            all_names = list(in_names) + list(out_names)
            if partition_name is not None:
                all_names.append(partition_name)

            def _body(*args, _nc=nc, _all=tuple(all_names),
                      _outs=tuple(out_names), _avals=tuple(out_avals),
                      _pn=partition_name):
                operands = list(args)
                if _pn is not None:
                    operands.append(bass2jax.partition_id_tensor())
                return tuple(bass2jax._bass_exec_p.bind(
                    *operands, out_avals=_avals, in_names=_all,
                    out_names=_outs, lowering_input_output_aliases=(),
                    sim_require_finite=True, sim_require_nnan=True, nc=_nc))

            n_params = len(in_names)
            donate = tuple(range(n_params, n_params + len(out_names)))
            _cache[key] = jax.jit(_body, donate_argnums=donate,
                                  keep_unused=True)
        fn = _cache[key]
        dev0 = jax.devices()[0]
        per_core = [jax.device_put(a.reshape(NCORES, a.shape[0] // NCORES,
                                             *a.shape[1:])[0], dev0)
                    for a in concat_in]
        for a in per_core:
            a.block_until_ready()
        times = []
        for _ in range(n):
            zeros = [jax.device_put(np.zeros(s, dt), dev0)
                     for s, dt in zero_shapes]
            for z in zeros:
                z.block_until_ready()
            t0 = time.perf_counter()
            o = fn(*per_core, *zeros)
            for x in o:
                x.block_until_ready()
            times.append(time.perf_counter() - t0)
        out[reps] = times
    return out


def bench_calibrated(n=6, hi_reps=3):
    """Time reps=1 vs reps=hi_reps executables; slope = true per-body time.
    Requires kernel() to have been called first (for cached inputs)."""
    import time
    import jax

    concat_in, zero_shapes = _cache["bench"]
    dev_in = [jax.device_put(a) for a in concat_in]
    for a in dev_in:
        a.block_until_ready()

    def time_exec(reps):
        sharded = _get_exec(reps)[0]
        times = []
        for _ in range(n):
            zeros = [jax.device_put(np.zeros((NCORES * s[0], *s[1:]), dt))
                     for s, dt in zero_shapes]
            for z in zeros:
                z.block_until_ready()
            t0 = time.perf_counter()
            out = sharded(*dev_in, *zeros)
            for o in out:
                o.block_until_ready()
            times.append(time.perf_counter() - t0)
        return times

    t1 = time_exec(1)
    tR = time_exec(hi_reps)
    per = (min(tR) - min(t1)) / (hi_reps - 1)
    return t1, tR, per


def _tokens(x):
    b = x.shape[0]
    t = x.reshape(b, C, NS, WS, NS, WS).transpose(0, 1, 2, 4, 3, 5)
    t = t.reshape(b, C, N, WS * WS).transpose(0, 2, 1, 3)
    return np.ascontiguousarray(t.reshape(b, N, TD))


def _untokens(o):
    b = o.shape[0]
    o = o.reshape(b, NS, NS, C, WS, WS).transpose(0, 3, 1, 4, 2, 5)
    return np.ascontiguousarray(o.reshape(b, C, IMG, IMG))


def _make_masks(s):
    # quad mask [g0 | g2 | g0 | g2] as [128, 512]; g0=triu (q>=p), g2=tril
    triu = np.triu(np.ones((W1, W1), np.float16))
    tril = np.tril(np.ones((W1, W1), np.float16))
    zer = np.zeros((W1, W1), np.float16)
    std = np.concatenate([triu, tril, triu, tril], axis=1)
    first = std.copy()
    last = std.copy()
    if s == 0:  # global chunk 0: its block-0 (g2 slot of kk=0) is invalid
        first[:, 128:256] = zer
        first[:, 384:512] = zer
    if s == SPLITS - 1:  # global chunk 127: block-2 (g0 slot of kk=33) invalid
        last[:, 0:128] = zer
        last[:, 256:384] = zer
    return (np.ascontiguousarray(std), np.ascontiguousarray(first),
            np.ascontiguousarray(last))


def kernel(**inputs):
    x = np.asarray(inputs["x"], dtype=np.float32)
    tokens = _tokens(x)  # (B, N, TD)
    scale = np.float32(1.0 / np.sqrt(D))

    host_w = {
        "wq": np.asarray(inputs["Wq"], np.float32) * scale,
        "wk": np.asarray(inputs["Wk"], np.float32),
        "wv": np.asarray(inputs["Wv"], np.float32),
        "wkg": np.asarray(inputs["Wkg"], np.float32),
        "wvg": np.asarray(inputs["Wvg"], np.float32),
        "wqg": np.asarray(inputs["Wqg"], np.float32) * scale,
    }
    host_w = {k: np.ascontiguousarray(v.astype(np.float16))
              for k, v in host_w.items()}
    host_b = {
        "bq": np.asarray(inputs["bq"], np.float32) * scale,
        "bqg": np.asarray(inputs["bqg"], np.float32) * scale,
    }
    host_b = {k: np.ascontiguousarray(v) for k, v in host_b.items()}
    # bv/bvg are added host-side after the softmax divide (probs sum to 1)
    bv_host = np.asarray(inputs["bv"], np.float32)
    bvg_host = np.asarray(inputs["bvg"], np.float32)

    in_maps = []
    for core in range(NCORES):
        b, s = divmod(core, SPLITS)
        lo = s * QLEN - HALO
        hi = (s + 1) * QLEN + HALO
        shard = np.zeros((NTOK, TD), np.float32)
        s0, s1 = max(lo, 0), min(hi, N)
        shard[s0 - lo:s1 - lo] = tokens[b, s0:s1]
        tokT = np.ascontiguousarray(shard.T.astype(np.float16))
        tokgT = np.ascontiguousarray(
            tokens[b, GPOS].T.astype(np.float16))
        m_std, m_first, m_last = _make_masks(s)
        m = dict(host_w)
        m.update(host_b)
        m["tokT"] = tokT
        m["tokgT"] = tokgT
        m["m_std"] = m_std
        m["m_first"] = m_first
        m["m_last"] = m_last
        in_maps.append(m)

    results = _run(in_maps)

    out = np.empty((B, N, TD), np.float32)
    og_acc = np.zeros((B, H, G, D + 1), np.float64)
    for core in range(NCORES):
        b, s = divmod(core, SPLITS)
        arr = results[core]["out_t"]  # (H, D+1, QLEN)
        o = arr[:, :D, :] / arr[:, D:D + 1, :]  # (H, D, QLEN)
        out[b, s * QLEN:(s + 1) * QLEN] = (
            o.transpose(2, 0, 1).reshape(QLEN, TD))
        og_acc[b] += results[core]["og_part"]
    out += bv_host  # bias deferred from the V projection
    og = (og_acc[..., :D] / og_acc[..., D:D + 1]).astype(np.float32)
    og = og.transpose(0, 2, 1, 3).reshape(B, G, TD)  # (B, G, H*D)
    og += bvg_host
    out[:, GPOS] = og
    return _untokens(out)

